# revision 5
# baseline (speedup 1.0000x reference)
"""Trainium2 Bass kernel for the interval-prediction custom loss.

total = 10*mean((t - (l+u)/2)^2) + 0.1*mean(u-l) + 10*mean(relu(l-u))
        + 0.5*sum(where(pv==0, relu(c-p), relu(p-c)))/N        with c=(l+u)/2

Strategy: pure data parallel over N across 8 NeuronCores. Each core reduces
its slice to [128, 5*n_tiles] partial sums (one column set per SBUF tile
pass); host does the tiny final reduction in float64.

Per-core per tile of FD elements/partition:
  DVE: h = l+u
       w = u-l                (fused accum -> sum w)
       e = 0.5h - t           (scalar_tensor_tensor)
       x = 0.5h - p           (scalar_tensor_tensor)
       q = x*v                (fused accum -> sum v*x)
  ACT: sum e^2  (Square accum), sum relu(-w), sum relu(x)
Identity used: relu((1-2v)*x) = relu(x) - v*x for v in {0,1}.
"""

import sys

if "/opt/trn_rl_repo" not in sys.path:
    sys.path.insert(0, "/opt/trn_rl_repo")

import numpy as np

N = 8388608
N_CORES = 8
P = 128
NP_PER_CORE = N // N_CORES            # 1048576
FPL = NP_PER_CORE // P                # 8192 elements per partition lane
N_TILES = 4
FD = FPL // N_TILES                   # 2048 elements per tile

_NC_CACHE = {}


def _build(fpl=FPL, n_tiles=N_TILES):
    """Build the per-core Bass program (identical on all cores)."""
    from concourse import bacc, mybir
    from concourse.tile import TileContext

    fd = fpl // n_tiles
    assert fd * n_tiles == fpl

    f32 = mybir.dt.float32
    Alu = mybir.AluOpType
    Act = mybir.ActivationFunctionType

    nc = bacc.Bacc(trn_type="TRN2")
    pred = nc.declare_dram_parameter("pred", [P, fpl, 2], f32, isOutput=False)
    targ = nc.declare_dram_parameter("targ", [P, fpl], f32, isOutput=False)
    prev = nc.declare_dram_parameter("prev", [P, fpl], f32, isOutput=False)
    pv = nc.declare_dram_parameter("pv", [P, fpl], f32, isOutput=False)
    out = nc.declare_dram_parameter("out", [P, 5 * n_tiles], f32, isOutput=True)

    with TileContext(nc) as tc:
        with (
            tc.tile_pool(name="io", bufs=2) as io_pool,
            tc.tile_pool(name="mid", bufs=2) as mid_pool,
            tc.tile_pool(name="acc", bufs=1) as acc_pool,
        ):
            acc_sq = acc_pool.tile([P, n_tiles], f32, tag="acc_sq")
            acc_w = acc_pool.tile([P, n_tiles], f32, tag="acc_w")
            acc_vd = acc_pool.tile([P, n_tiles], f32, tag="acc_vd")
            acc_rx = acc_pool.tile([P, n_tiles], f32, tag="acc_rx")
            acc_vx = acc_pool.tile([P, n_tiles], f32, tag="acc_vx")

            for j in range(n_tiles):
                pred_t = io_pool.tile([P, fd, 2], f32, tag="pred")
                t_t = io_pool.tile([P, fd], f32, tag="t")
                p_t = io_pool.tile([P, fd], f32, tag="p")
                v_t = io_pool.tile([P, fd], f32, tag="v")
                nc.sync.dma_start(out=pred_t, in_=pred[:, j * fd : (j + 1) * fd, :])
                nc.sync.dma_start(out=t_t, in_=targ[:, j * fd : (j + 1) * fd])
                nc.sync.dma_start(out=p_t, in_=prev[:, j * fd : (j + 1) * fd])
                nc.sync.dma_start(out=v_t, in_=pv[:, j * fd : (j + 1) * fd])

                lo = pred_t[:, :, 0]
                up = pred_t[:, :, 1]

                h = mid_pool.tile([P, fd], f32, tag="h")
                w = mid_pool.tile([P, fd], f32, tag="w")
                e = mid_pool.tile([P, fd], f32, tag="e")
                x = mid_pool.tile([P, fd], f32, tag="x")
                jd = mid_pool.tile([P, fd], f32, tag="jd")
                ja = mid_pool.tile([P, fd], f32, tag="ja")

                # h = lo + up
                nc.vector.tensor_add(out=h, in0=lo, in1=up)
                # w = up - lo ; acc_w[:, j] = sum(w)
                nc.vector.scalar_tensor_tensor(
                    out=w, in0=up, scalar=1.0, in1=lo,
                    op0=Alu.mult, op1=Alu.subtract,
                    accum_out=acc_w[:, j : j + 1],
                )
                # e = 0.5*h - t   (== -(t - center))
                nc.vector.scalar_tensor_tensor(
                    out=e, in0=h, scalar=0.5, in1=t_t,
                    op0=Alu.mult, op1=Alu.subtract,
                )
                # x = 0.5*h - p   (== center - prev)
                nc.vector.scalar_tensor_tensor(
                    out=x, in0=h, scalar=0.5, in1=p_t,
                    op0=Alu.mult, op1=Alu.subtract,
                )
                # acc_vx[:, j] = sum(x * v)
                nc.vector.scalar_tensor_tensor(
                    out=jd, in0=x, scalar=1.0, in1=v_t,
                    op0=Alu.mult, op1=Alu.mult,
                    accum_out=acc_vx[:, j : j + 1],
                )
                # acc_sq[:, j] = sum(e^2)
                nc.scalar.activation(
                    out=ja, in_=e, func=Act.Square,
                    accum_out=acc_sq[:, j : j + 1],
                )
                # acc_vd[:, j] = sum(relu(-w)) = sum(relu(lo - up))
                nc.scalar.activation(
                    out=ja, in_=w, func=Act.Relu, scale=-1.0,
                    accum_out=acc_vd[:, j : j + 1],
                )
                # acc_rx[:, j] = sum(relu(x))
                nc.scalar.activation(
                    out=ja, in_=x, func=Act.Relu,
                    accum_out=acc_rx[:, j : j + 1],
                )

            nc.sync.dma_start(out=out[:, 0 * n_tiles : 1 * n_tiles], in_=acc_sq)
            nc.sync.dma_start(out=out[:, 1 * n_tiles : 2 * n_tiles], in_=acc_w)
            nc.sync.dma_start(out=out[:, 2 * n_tiles : 3 * n_tiles], in_=acc_vd)
            nc.sync.dma_start(out=out[:, 3 * n_tiles : 4 * n_tiles], in_=acc_rx)
            nc.sync.dma_start(out=out[:, 4 * n_tiles : 5 * n_tiles], in_=acc_vx)

    nc.compile()
    return nc


def _get_nc():
    key = (FPL, N_TILES)
    if key not in _NC_CACHE:
        _NC_CACHE[key] = _build()
    return _NC_CACHE[key]


def _shard(inputs):
    pred = np.ascontiguousarray(np.asarray(inputs["pred"]).astype(np.float32, copy=False))
    targ = np.asarray(inputs["target"]).astype(np.float32, copy=False).reshape(N)
    prev = np.asarray(inputs["prev_pci"]).astype(np.float32, copy=False).reshape(N)
    # int64 is unsupported on-device; values are 0/1 so a f32 cast is exact.
    pv = np.asarray(inputs["pv_values"]).astype(np.float32).reshape(N)

    in_maps = []
    for c in range(N_CORES):
        s = slice(c * NP_PER_CORE, (c + 1) * NP_PER_CORE)
        in_maps.append(
            {
                "pred": np.ascontiguousarray(pred[s].reshape(P, FPL, 2)),
                "targ": np.ascontiguousarray(targ[s].reshape(P, FPL)),
                "prev": np.ascontiguousarray(prev[s].reshape(P, FPL)),
                "pv": np.ascontiguousarray(pv[s].reshape(P, FPL)),
            }
        )
    return in_maps


def _combine(core_outs, n_tiles=N_TILES, n=N):
    """core_outs: list of [P, 5*n_tiles] partial-sum arrays."""
    allp = np.stack([np.asarray(o, dtype=np.float64) for o in core_outs])
    s = allp.reshape(len(core_outs), P, 5, n_tiles).sum(axis=(0, 1, 3))
    s_sq, s_w, s_vd, s_rx, s_vx = s
    center_loss = s_sq / n
    width_loss = s_w / n
    valid_penalty = s_vd / n
    direction_penalty = s_rx - s_vx
    total = (
        center_loss * 10.0
        + 0.1 * width_loss
        + 10.0 * valid_penalty
        + 0.5 * direction_penalty / n
    )
    return np.array(total, dtype=np.float32)


def _run(inputs, trace=False):
    """Run the SPMD kernel; returns (scalar_result, BassKernelResults)."""
    from concourse.bass_utils import run_bass_kernel_spmd

    nc = _get_nc()
    in_maps = _shard(inputs)
    res = run_bass_kernel_spmd(
        nc, in_maps, core_ids=list(range(N_CORES)), trace=trace
    )
    core_outs = [res.results[c]["out"] for c in range(N_CORES)]
    return _combine(core_outs), res


def kernel(**inputs) -> np.ndarray:
    result, _ = _run(inputs, trace=False)
    return result


# revision 9
# speedup vs baseline: 1.0228x; 1.0228x over previous
"""Trainium2 Bass kernel for the interval-prediction custom loss.

total = 10*mean((t - (l+u)/2)^2) + 0.1*mean(u-l) + 10*mean(relu(l-u))
        + 0.5*sum(where(pv==0, relu(c-p), relu(p-c)))/N        with c=(l+u)/2

Strategy: pure data parallel over N across 8 NeuronCores. Each core reduces
its slice to [128, 5*n_tiles] partial sums (one column set per SBUF tile
pass); host does the tiny final reduction in float64.

Per-core per tile of FD elements/partition:
  DVE: h = l+u
       w = u-l                (fused accum -> sum w)
       e = 0.5h - t           (scalar_tensor_tensor)
       x = 0.5h - p           (scalar_tensor_tensor)
       q = x*v                (fused accum -> sum v*x)
  ACT: sum e^2  (Square accum), sum relu(-w), sum relu(x)
Identity used: relu((1-2v)*x) = relu(x) - v*x for v in {0,1}.
"""

import sys

if "/opt/trn_rl_repo" not in sys.path:
    sys.path.insert(0, "/opt/trn_rl_repo")

import numpy as np

N = 8388608
N_CORES = 8
P = 128
NP_PER_CORE = N // N_CORES            # 1048576
FPL = NP_PER_CORE // P                # 8192 elements per partition lane
N_TILES = 8
FD = FPL // N_TILES                   # 1024 elements per tile

_NC_CACHE = {}


def _build(fpl=FPL, n_tiles=N_TILES):
    """Build the per-core Bass program (identical on all cores)."""
    from concourse import bacc, mybir
    from concourse.tile import TileContext

    fd = fpl // n_tiles
    assert fd * n_tiles == fpl

    f32 = mybir.dt.float32
    u8 = mybir.dt.uint8
    Alu = mybir.AluOpType
    Act = mybir.ActivationFunctionType

    nc = bacc.Bacc(trn_type="TRN2")
    pred = nc.declare_dram_parameter("pred", [P, fpl, 2], f32, isOutput=False)
    targ = nc.declare_dram_parameter("targ", [P, fpl], f32, isOutput=False)
    prev = nc.declare_dram_parameter("prev", [P, fpl], f32, isOutput=False)
    pv = nc.declare_dram_parameter("pv", [P, fpl], u8, isOutput=False)
    out = nc.declare_dram_parameter("out", [P, 5 * n_tiles], f32, isOutput=True)

    with TileContext(nc) as tc:
        with (
            tc.tile_pool(name="io", bufs=3) as io_pool,
            tc.tile_pool(name="mid", bufs=2) as mid_pool,
            tc.tile_pool(name="acc", bufs=1) as acc_pool,
        ):
            acc_sq = acc_pool.tile([P, n_tiles], f32, tag="acc_sq")
            acc_w = acc_pool.tile([P, n_tiles], f32, tag="acc_w")
            acc_vd = acc_pool.tile([P, n_tiles], f32, tag="acc_vd")
            acc_rx = acc_pool.tile([P, n_tiles], f32, tag="acc_rx")
            acc_vx = acc_pool.tile([P, n_tiles], f32, tag="acc_vx")

            for j in range(n_tiles):
                pred_t = io_pool.tile([P, fd, 2], f32, tag="pred")
                t_t = io_pool.tile([P, fd], f32, tag="t")
                p_t = io_pool.tile([P, fd], f32, tag="p")
                v_t = io_pool.tile([P, fd], u8, tag="v")
                nc.sync.dma_start(out=pred_t, in_=pred[:, j * fd : (j + 1) * fd, :])
                nc.sync.dma_start(out=t_t, in_=targ[:, j * fd : (j + 1) * fd])
                nc.sync.dma_start(out=p_t, in_=prev[:, j * fd : (j + 1) * fd])
                nc.sync.dma_start(out=v_t, in_=pv[:, j * fd : (j + 1) * fd])

                lo = pred_t[:, :, 0]
                up = pred_t[:, :, 1]

                h = mid_pool.tile([P, fd], f32, tag="h")
                w = mid_pool.tile([P, fd], f32, tag="w")
                e = mid_pool.tile([P, fd], f32, tag="e")
                x = mid_pool.tile([P, fd], f32, tag="x")
                jd = mid_pool.tile([P, fd], f32, tag="jd")
                ja = mid_pool.tile([P, fd], f32, tag="ja")

                # h = lo + up
                nc.vector.tensor_add(out=h, in0=lo, in1=up)
                # w = up - lo ; acc_w[:, j] = sum(w)
                nc.vector.scalar_tensor_tensor(
                    out=w, in0=up, scalar=1.0, in1=lo,
                    op0=Alu.mult, op1=Alu.subtract,
                    accum_out=acc_w[:, j : j + 1],
                )
                # e = 0.5*h - t   (== -(t - center))
                nc.vector.scalar_tensor_tensor(
                    out=e, in0=h, scalar=0.5, in1=t_t,
                    op0=Alu.mult, op1=Alu.subtract,
                )
                # x = 0.5*h - p   (== center - prev)
                nc.vector.scalar_tensor_tensor(
                    out=x, in0=h, scalar=0.5, in1=p_t,
                    op0=Alu.mult, op1=Alu.subtract,
                )
                # acc_vx[:, j] = sum(x * v)
                nc.vector.scalar_tensor_tensor(
                    out=jd, in0=x, scalar=1.0, in1=v_t,
                    op0=Alu.mult, op1=Alu.mult,
                    accum_out=acc_vx[:, j : j + 1],
                )
                # acc_sq[:, j] = sum(e^2)
                nc.scalar.activation(
                    out=ja, in_=e, func=Act.Square,
                    accum_out=acc_sq[:, j : j + 1],
                )
                # acc_vd[:, j] = sum(relu(-w)) = sum(relu(lo - up))
                nc.scalar.activation(
                    out=ja, in_=w, func=Act.Relu, scale=-1.0,
                    accum_out=acc_vd[:, j : j + 1],
                )
                # acc_rx[:, j] = sum(relu(x))
                nc.scalar.activation(
                    out=ja, in_=x, func=Act.Relu,
                    accum_out=acc_rx[:, j : j + 1],
                )

            nc.sync.dma_start(out=out[:, 0 * n_tiles : 1 * n_tiles], in_=acc_sq)
            nc.sync.dma_start(out=out[:, 1 * n_tiles : 2 * n_tiles], in_=acc_w)
            nc.sync.dma_start(out=out[:, 2 * n_tiles : 3 * n_tiles], in_=acc_vd)
            nc.sync.dma_start(out=out[:, 3 * n_tiles : 4 * n_tiles], in_=acc_rx)
            nc.sync.dma_start(out=out[:, 4 * n_tiles : 5 * n_tiles], in_=acc_vx)

    nc.compile()
    return nc


def _get_nc():
    key = (FPL, N_TILES)
    if key not in _NC_CACHE:
        _NC_CACHE[key] = _build()
    return _NC_CACHE[key]


def _shard(inputs):
    pred = np.ascontiguousarray(np.asarray(inputs["pred"]).astype(np.float32, copy=False))
    targ = np.asarray(inputs["target"]).astype(np.float32, copy=False).reshape(N)
    prev = np.asarray(inputs["prev_pci"]).astype(np.float32, copy=False).reshape(N)
    # int64 is unsupported on-device; values are 0/1 so a uint8 cast is exact.
    pv = np.asarray(inputs["pv_values"]).astype(np.uint8).reshape(N)

    in_maps = []
    for c in range(N_CORES):
        s = slice(c * NP_PER_CORE, (c + 1) * NP_PER_CORE)
        in_maps.append(
            {
                "pred": np.ascontiguousarray(pred[s].reshape(P, FPL, 2)),
                "targ": np.ascontiguousarray(targ[s].reshape(P, FPL)),
                "prev": np.ascontiguousarray(prev[s].reshape(P, FPL)),
                "pv": np.ascontiguousarray(pv[s].reshape(P, FPL)),
            }
        )
    return in_maps


def _combine(core_outs, n_tiles=N_TILES, n=N):
    """core_outs: list of [P, 5*n_tiles] partial-sum arrays."""
    allp = np.stack([np.asarray(o, dtype=np.float64) for o in core_outs])
    s = allp.reshape(len(core_outs), P, 5, n_tiles).sum(axis=(0, 1, 3))
    s_sq, s_w, s_vd, s_rx, s_vx = s
    center_loss = s_sq / n
    width_loss = s_w / n
    valid_penalty = s_vd / n
    direction_penalty = s_rx - s_vx
    total = (
        center_loss * 10.0
        + 0.1 * width_loss
        + 10.0 * valid_penalty
        + 0.5 * direction_penalty / n
    )
    return np.array(total, dtype=np.float32)


def _run(inputs, trace=False):
    """Run the SPMD kernel; returns (scalar_result, BassKernelResults)."""
    from concourse.bass_utils import run_bass_kernel_spmd

    nc = _get_nc()
    in_maps = _shard(inputs)
    res = run_bass_kernel_spmd(
        nc, in_maps, core_ids=list(range(N_CORES)), trace=trace
    )
    core_outs = [res.results[c]["out"] for c in range(N_CORES)]
    return _combine(core_outs), res


def kernel(**inputs) -> np.ndarray:
    result, _ = _run(inputs, trace=False)
    return result
